# revision 1
# baseline (speedup 1.0000x reference)
"""Trainium2 Bass kernel for nn_Attention_53798760350139.

Module: x + pos_enc -> unscaled self-attention (softmax(x x^T) x) -> MLP ->
residual -> full-sample layernorm.  B=16, H=W=48, D=384.

Sharding: data-parallel over batch across 8 cores (2 batches per core),
weights replicated.  Inputs are FULL tensors; output is the FULL tensor.
"""
import numpy as np
from contextlib import ExitStack

import concourse.bass as bass
import concourse.tile as tile
from concourse import bacc, mybir
from concourse.bass_utils import run_bass_kernel_spmd
from concourse.masks import make_identity
from concourse.bass import ts

F32 = mybir.dt.float32
F32R = mybir.dt.float32r
BF16 = mybir.dt.bfloat16
F16 = mybir.dt.float16

B, H, W, D = 16, 48, 48, 384
NT = H * W          # 2304 tokens
NCORES = 8
BPC = B // NCORES   # 2 batches per core
KT = D // 128       # 3 contraction tiles over D
TB = NT // 128      # 18 token blocks
CH = 256            # i-chunk width for AV/MLP stages
NCH = NT // CH      # 9 chunks
IBC = CH // 128     # 2 i-blocks per chunk
JT = [(0, 512), (512, 512), (1024, 512), (1536, 512), (2048, 256)]
EPS = 1e-5

_prog_cache = {}


def _build_program():
    nc = bacc.Bacc("TRN2", target_bir_lowering=False, debug=False)

    xp_d = nc.dram_tensor("xp", [BPC, NT, D], F32, kind="ExternalInput").ap()
    w1_d = nc.dram_tensor("w1", [D, D], F32, kind="ExternalInput").ap()
    w2_d = nc.dram_tensor("w2", [D, D], F32, kind="ExternalInput").ap()
    b1_d = nc.dram_tensor("b1", [D, 1], F32, kind="ExternalInput").ap()
    b2b_d = nc.dram_tensor("b2b", [128, D], F32, kind="ExternalInput").ap()
    out_d = nc.dram_tensor("out", [BPC, NT, D], F32, kind="ExternalOutput").ap()

    with tile.TileContext(nc) as tc, ExitStack() as ctx:
        const = ctx.enter_context(tc.tile_pool(name="const", bufs=1))
        sbig = ctx.enter_context(tc.tile_pool(name="sbig", bufs=1))
        xn_pool = ctx.enter_context(tc.tile_pool(name="xn", bufs=2))
        s_pool = ctx.enter_context(tc.tile_pool(name="s", bufs=2))
        pu_pool = ctx.enter_context(tc.tile_pool(name="pu", bufs=2))
        pT_pool = ctx.enter_context(tc.tile_pool(name="pT", bufs=2))
        oT_pool = ctx.enter_context(tc.tile_pool(name="oT", bufs=2))
        hT_pool = ctx.enter_context(tc.tile_pool(name="hT", bufs=2))
        small = ctx.enter_context(tc.tile_pool(name="small", bufs=6))
        scr_pool = ctx.enter_context(tc.tile_pool(name="scr", bufs=1))
        ps512 = ctx.enter_context(tc.tile_pool(name="ps512", bufs=3, space="PSUM"))
        psml = ctx.enter_context(tc.tile_pool(name="psml", bufs=2, space="PSUM"))
        pstr = ctx.enter_context(tc.tile_pool(name="pstr", bufs=1, space="PSUM"))

        # ---------- constants / weights ----------
        ident = const.tile([128, 128], F32, tag="ident")
        make_identity(nc, ident[:])
        ident16 = const.tile([128, 128], F16, tag="ident16")
        make_identity(nc, ident16[:])
        ones_col = const.tile([128, 1], F32, tag="ones_col")
        nc.vector.memset(ones_col[:], 1.0)
        ones_row = const.tile([1, 128], F32, tag="ones_row")
        nc.vector.memset(ones_row[:], 1.0)

        w1f = const.tile([128, KT, D], F32, tag="w1f")
        w2f = const.tile([128, KT, D], F32, tag="w2f")
        nc.sync.dma_start(w1f[:], w1_d.rearrange("(t p) m -> p t m", p=128))
        nc.sync.dma_start(w2f[:], w2_d.rearrange("(t p) m -> p t m", p=128))
        w1r = const.tile([128, KT, D], F32R, tag="w1r")
        w2r = const.tile([128, KT, D], F32R, tag="w2r")
        nc.vector.tensor_copy(w1r[:], w1f[:])
        nc.vector.tensor_copy(w2r[:], w2f[:])
        b1_t = const.tile([128, KT, 1], F32, tag="b1t")
        nc.sync.dma_start(b1_t[:], b1_d.rearrange("(t p) o -> p t o", p=128))
        b2b_t = const.tile([128, D], F32, tag="b2bt")
        nc.sync.dma_start(b2b_t[:], b2b_d)

        for b in range(BPC):
            # ---------- stage 0: load this batch ----------
            xnat = xn_pool.tile([128, TB, D], F32, tag="xnat")
            nc.sync.dma_start(xnat[:], xp_d[b].rearrange("(t p) d -> p t d", p=128))
            xf16 = sbig.tile([128, TB, D], F16, tag="xf16")
            nc.vector.tensor_copy(xf16[:], xnat[:])

            # ---------- stage 1: transpose to [d, token] + precision splits
            xr = sbig.tile([128, KT, NT], F32R, tag="xr")
            xe = sbig.tile([128, KT, NT], BF16, tag="xe")
            xb = sbig.tile([128, KT, NT], BF16, tag="xb")
            for t in range(TB):
                for k in range(KT):
                    tp = ps512.tile([128, 512], F32, tag="ps512")
                    nc.tensor.transpose(
                        tp[:, :128], xnat[:, t, ts(k, 128)], ident[:]
                    )
                    nc.scalar.copy(xr[:, k, ts(t, 128)], tp[:, :128])
                    nc.vector.tensor_tensor(
                        xe[:, k, ts(t, 128)],
                        tp[:, :128],
                        xr[:, k, ts(t, 128)].bitcast(F32),
                        mybir.AluOpType.subtract,
                    )
                    nc.vector.tensor_copy(xb[:, k, ts(t, 128)], tp[:, :128])

            # LN stats accumulators
            stats = sbig.tile([128, 2, TB], F32, tag="stats")

            # ---------- stages 2+3: software-pipelined i-block loop ----------
            # slot ib: scores+softmax(ib); transposes(ib-1); after transposes
            # of a chunk's last block, that chunk's AV+MLP tail.
            pT_bufs = {}

            def emit_scores_softmax(ib):
                s_t = s_pool.tile([128, NT], F32, tag="s")
                pmax = small.tile([128, len(JT)], F32, tag="pmax")
                for tj, (off, w) in enumerate(JT):
                    acc = ps512.tile([128, 512], F32, tag="ps512")
                    i_mm = 0
                    for k in range(KT):
                        for lhs, rhs in (
                            (xr[:, k, ts(ib, 128)], xr[:, k, off : off + w]),
                            (xe[:, k, ts(ib, 128)], xb[:, k, off : off + w]),
                            (xb[:, k, ts(ib, 128)], xe[:, k, off : off + w]),
                        ):
                            nc.tensor.matmul(
                                acc[:, :w], lhs, rhs,
                                start=(i_mm == 0), stop=(i_mm == 3 * KT - 1),
                            )
                            i_mm += 1
                    nc.scalar.copy(s_t[:, off : off + w], acc[:, :w])
                    # partial row max straight from PSUM (off critical tail)
                    nc.vector.tensor_reduce(
                        pmax[:, tj : tj + 1], acc[:, :w],
                        axis=mybir.AxisListType.X, op=mybir.AluOpType.max,
                    )
                mneg = small.tile([128, 1], F32, tag="mneg")
                nc.vector.tensor_reduce(
                    mneg[:], pmax[:], axis=mybir.AxisListType.X,
                    op=mybir.AluOpType.max, negate=True,
                )
                pu = pu_pool.tile([128, NT], F16, tag="pu")
                l_t = small.tile([128, 1], F32, tag="l")
                nc.scalar.activation(
                    pu[:], s_t[:], mybir.ActivationFunctionType.Exp,
                    bias=mneg[:], scale=1.0, accum_out=l_t[:],
                )
                r_t = small.tile([128, 1], F32, tag="r")
                nc.vector.reciprocal(r_t[:], l_t[:])
                nc.vector.tensor_scalar_mul(pu[:], pu[:], r_t[:])
                return pu

            def emit_transposes(ib, pf):
                c, ibl = divmod(ib, IBC)
                if ibl == 0:
                    buf = pT_pool.tile([128, TB, CH], F16, tag="pT")
                    pT_bufs[c] = buf
                pT_buf = pT_bufs[c]
                for g, gw in ((0, 8), (1, 8), (2, 2)):
                    tps = pstr.tile([128, 8, 128], F16, tag="tps")
                    for jj in range(gw):
                        jt = g * 8 + jj
                        nc.tensor.transpose(
                            tps[:, jj, :], pf[:, ts(jt, 128)], ident16[:]
                        )
                    nc.vector.tensor_copy(
                        pT_buf[:, g * 8 : g * 8 + gw, ts(ibl, 128)],
                        tps[:, :gw, :],
                    )

            def emit_chunk_tail(c):
                pT_buf = pT_bufs.pop(c)
                # AV: oT[d, i_chunk] = sum_j x[j, d] p[i, j]
                oacc = psml.tile([128, KT, CH], F32, tag="psml")
                for dm in range(KT):
                    for j in range(TB):
                        nc.tensor.matmul(
                            oacc[:, dm, :],
                            xf16[:, j, ts(dm, 128)],
                            pT_buf[:, j, :],
                            start=(j == 0),
                            stop=(j == TB - 1),
                        )
                oT = oT_pool.tile([128, KT, CH], F32R, tag="oT")
                nc.vector.tensor_copy(oT[:], oacc[:, :, :CH])

                # MLP layer 1 (transposed layout): hT = relu(W1^T oT + b1)
                hacc = psml.tile([128, KT, CH], F32, tag="psml")
                for dm in range(KT):
                    for k in range(KT):
                        nc.tensor.matmul(
                            hacc[:, dm, :],
                            w1r[:, k, ts(dm, 128)],
                            oT[:, k, :],
                            start=(k == 0),
                            stop=(k == KT - 1),
                        )
                hT = hT_pool.tile([128, KT, CH], F32R, tag="hT")
                for dm in range(KT):
                    nc.scalar.activation(
                        hT[:, dm, :], hacc[:, dm, :],
                        mybir.ActivationFunctionType.Relu,
                        bias=b1_t[:, dm, :], scale=1.0,
                    )

                # MLP layer 2 in natural layout + residual + b2
                for ibl in range(IBC):
                    ib = c * IBC + ibl
                    acc2 = ps512.tile([128, 512], F32, tag="ps512")
                    for k in range(KT):
                        nc.tensor.matmul(
                            acc2[:, :D],
                            hT[:, k, ts(ibl, 128)],
                            w2r[:, k, :],
                            start=(k == 0),
                            stop=(k == KT - 1),
                        )
                    nc.vector.tensor_tensor(
                        xnat[:, ib, :], acc2[:, :D], xnat[:, ib, :],
                        mybir.AluOpType.add,
                    )
                    nc.vector.tensor_tensor(
                        xnat[:, ib, :], xnat[:, ib, :], b2b_t[:],
                        mybir.AluOpType.add,
                    )
                    # LN partial stats for this block
                    scr = scr_pool.tile([128, D], F32, tag="scr")
                    nc.vector.tensor_scalar(
                        scr[:], xnat[:, ib, :], 0.0, 0.0,
                        mybir.AluOpType.add, mybir.AluOpType.add,
                        accum_out=stats[:, 0, ib : ib + 1],
                    )
                    scr2 = scr_pool.tile([128, D], F32, tag="scr")
                    nc.vector.scalar_tensor_tensor(
                        scr2[:], xnat[:, ib, :], 1.0, xnat[:, ib, :],
                        mybir.AluOpType.mult, mybir.AluOpType.mult,
                        accum_out=stats[:, 1, ib : ib + 1],
                    )

            pf_prev = None
            for ib in range(TB):
                pf_cur = emit_scores_softmax(ib)
                if pf_prev is not None:
                    emit_transposes(ib - 1, pf_prev)
                    if ib >= 2 and ib % IBC == 0:
                        emit_chunk_tail(ib // IBC - 1)
                pf_prev = pf_cur
            emit_transposes(TB - 1, pf_prev)
            emit_chunk_tail(NCH - 1)

            # ---------- layernorm finalize ----------
            pstat = ps512.tile([128, 512], F32, tag="ps512")
            nc.tensor.matmul(
                pstat[:1, : 2 * TB],
                ones_col[:],
                stats[:].rearrange("p a b -> p (a b)"),
                start=True,
                stop=True,
            )
            tot = small.tile([1, 2], F32, tag="tot")
            nc.vector.tensor_reduce(
                tot[:],
                pstat[:1, : 2 * TB].rearrange("p (a b) -> p a b", a=2),
                axis=mybir.AxisListType.X,
                op=mybir.AluOpType.add,
            )
            NALL = float(NT * D)
            mv = small.tile([1, 2], F32, tag="mv")  # [mean, e2]
            nc.vector.tensor_scalar_mul(mv[:], tot[:], 1.0 / NALL)
            msq = small.tile([1, 1], F32, tag="msq")
            nc.vector.tensor_tensor(
                msq[:], mv[:, :1], mv[:, :1], mybir.AluOpType.mult
            )
            vare = small.tile([1, 1], F32, tag="vare")
            nc.vector.tensor_tensor(
                vare[:], mv[:, 1:2], msq[:], mybir.AluOpType.subtract
            )
            nc.vector.tensor_scalar_add(vare[:], vare[:], EPS)
            sd = small.tile([1, 1], F32, tag="sd")
            nc.scalar.sqrt(sd[:], vare[:])
            r0 = small.tile([1, 1], F32, tag="r0")
            nc.vector.reciprocal(r0[:], sd[:])
            # one Newton step for rsqrt accuracy: r1 = r0*(1.5 - 0.5*vare*r0^2)
            t_a = small.tile([1, 1], F32, tag="ta")
            nc.vector.tensor_tensor(t_a[:], r0[:], r0[:], mybir.AluOpType.mult)
            nc.vector.tensor_tensor(t_a[:], t_a[:], vare[:], mybir.AluOpType.mult)
            nc.vector.tensor_scalar(
                t_a[:], t_a[:], -0.5, 1.5, mybir.AluOpType.mult, mybir.AluOpType.add
            )
            r1 = small.tile([1, 1], F32, tag="r1")
            nc.vector.tensor_tensor(r1[:], r0[:], t_a[:], mybir.AluOpType.mult)
            # broadcast mean and r1 to all partitions
            mr = small.tile([1, 2], F32, tag="mr")
            nc.vector.tensor_copy(mr[:, :1], mv[:, :1])
            nc.vector.tensor_copy(mr[:, 1:2], r1[:])
            pbc = ps512.tile([128, 512], F32, tag="ps512")
            nc.tensor.matmul(
                pbc[:, :2], ones_row[:], mr[:], start=True, stop=True
            )
            mrb = small.tile([128, 2], F32, tag="mrb")
            nc.vector.tensor_copy(mrb[:], pbc[:, :2])
            # normalize in place and store
            for ib in range(TB):
                nc.vector.tensor_scalar(
                    xnat[:, ib, :], xnat[:, ib, :],
                    mrb[:, 0:1], mrb[:, 1:2],
                    mybir.AluOpType.subtract, mybir.AluOpType.mult,
                )
            nc.sync.dma_start(
                out_d[b].rearrange("(t p) d -> p t d", p=128), xnat[:]
            )

    nc.compile()
    return nc


def _host_prep(x, Wp, bp, b2):
    ph = np.arange(H, dtype=np.float32)[:, None] * np.ones((1, W), np.float32)
    pw = np.arange(W, dtype=np.float32)[None, :] * np.ones((H, 1), np.float32)
    pos = np.stack((ph, pw), axis=-1).reshape(NT, 2)
    pos_enc = pos @ Wp.astype(np.float32) + bp.astype(np.float32)
    xp = x.reshape(B, NT, D).astype(np.float32) + pos_enc[None]
    b2b = np.broadcast_to(b2.astype(np.float32), (128, D)).copy()
    return xp, b2b


def kernel(x, Wp, bp, W1, b1, W2, b2):
    x = np.asarray(x, dtype=np.float32)
    Wp = np.asarray(Wp, dtype=np.float32)
    bp = np.asarray(bp, dtype=np.float32)
    W1 = np.asarray(W1, dtype=np.float32)
    b1 = np.asarray(b1, dtype=np.float32)
    W2 = np.asarray(W2, dtype=np.float32)
    b2 = np.asarray(b2, dtype=np.float32)

    xp, b2b = _host_prep(x, Wp, bp, b2)

    if "nc" not in _prog_cache:
        _prog_cache["nc"] = _build_program()
    nc = _prog_cache["nc"]

    in_maps = []
    for core in range(NCORES):
        in_maps.append(
            {
                "xp": np.ascontiguousarray(xp[core * BPC : (core + 1) * BPC]),
                "w1": W1,
                "w2": W2,
                "b1": np.ascontiguousarray(b1[:, None]),
                "b2b": b2b,
            }
        )
    res = run_bass_kernel_spmd(nc, in_maps, core_ids=list(range(NCORES)))
    _prog_cache["last_results"] = res
    out = np.concatenate([r["out"] for r in res.results], axis=0)
    return out.reshape(B, H, W, D).astype(np.float32)

